# revision 7
# baseline (speedup 1.0000x reference)
"""Trainium2 Bass kernel for full-embed-dim self-attention + residual LayerNorm.

Problem: B=4, S=2048, D=1024 fp32.
  q/k/v = x@w{q,k,v}+b; scores = q@k^T/sqrt(D); attn = softmax(scores)@v;
  out = LN(x + attn@wo + bo) * gamma + beta.

Sharding: 8 cores = 4 batches x 2 query-halves (1024 queries each). Each
core computes K/V projections only for its own 1024 keys, then a pair-wise
AllGather ([0,1],[2,3],...) exchanges the halves so each core attends over
the batch's full 2048-key sequence.

Per-core dataflow (d-on-partitions "transposed" layout throughout):
  QT[d,q]   = wq^T @ xq^T            (host ships x^T for the core's half)
  KT[d,k_own] = wk^T @ xq^T (+bk per-partition)   --> AllGather --> KT full
  V[k_own,d]  = xq^T^T @ wv                       --> AllGather --> V full
  ST[k,q]   = KT^T @ QT              (contracted over d in PSUM)
  PT[k,q]   = exp(ST/sqrt(D) + kbias[k]),  kbias = K@(bq/sqrt(D))
              (per-q softmax factors cancel after normalization; logits are
               O(5) std-1 so exp needs no max-subtraction in fp32)
  denom[q]  = ones^T @ PT            (PE column-sum)
  AT[d,q]   = V^T @ PT
  O[q,e]    = AT^T @ wo, then O/denom[q] + xq_aug (host adds bo+bv@wo), LN.

Matmul inputs are bf16 (1 PE cycle/row vs 4 for fp32; fp32 PSUM accum).
"""

import numpy as np
import ml_dtypes

import concourse.bass as bass
import concourse.mybir as mybir
import concourse.tile as tile
from concourse import bacc

F32 = mybir.dt.float32
BF16 = mybir.dt.bfloat16

B, S, D = 4, 2048, 1024
Q = 1024          # queries (and own keys) per core
SCALE = 1.0 / 32.0
EPS = 1e-6
NKC = S // 128    # 16 key chunks (global)
NDC = D // 128    # 8 d chunks
RG = [[0, 1], [2, 3], [4, 5], [6, 7]]


def _bcast_ap(ap_1d, parts=128):
    """[N] dram AP -> [parts, N] AP with 0-stride partition dim."""
    return bass.AP(
        tensor=ap_1d.tensor, offset=ap_1d.offset, ap=[[0, parts]] + list(ap_1d.ap)
    )


def build_nc():
    nc = bacc.Bacc("TRN2", target_bir_lowering=False, debug=False, num_devices=8)

    xqt = nc.dram_tensor("xqt", [D, Q], BF16, kind="ExternalInput")
    xq = nc.dram_tensor("xq", [Q, D], F32, kind="ExternalInput")
    wq_d = nc.dram_tensor("wq", [D, D], BF16, kind="ExternalInput")
    wk_d = nc.dram_tensor("wk", [D, D], BF16, kind="ExternalInput")
    wv_d = nc.dram_tensor("wv", [D, D], BF16, kind="ExternalInput")
    wo_d = nc.dram_tensor("wo", [D, D], BF16, kind="ExternalInput")
    bqT_d = nc.dram_tensor("bqT", [128, NDC], F32, kind="ExternalInput")
    bkT_d = nc.dram_tensor("bkT", [128, NDC], F32, kind="ExternalInput")
    gamma_d = nc.dram_tensor("gamma", [D], F32, kind="ExternalInput")
    beta_d = nc.dram_tensor("beta", [D], F32, kind="ExternalInput")
    out_d = nc.dram_tensor("out", [Q, D], F32, kind="ExternalOutput")

    with tile.TileContext(nc) as tc:
        with (
            tc.tile_pool(name="small", bufs=1) as p_small,
            tc.tile_pool(name="dram", bufs=1, space="DRAM") as p_dram,
            tc.tile_pool(name="qtsb", bufs=NDC) as p_qt,
            tc.tile_pool(name="ptsb", bufs=NKC) as p_pt,
            tc.tile_pool(name="ps", bufs=6, space="PSUM") as p_ps,
            tc.tile_pool(name="ps1", bufs=2, space="PSUM") as p_ps1,
        ):
            kvin_kt = [p_dram.tile([Q // 2, D], BF16, name=f"kvin_kt{i}") for i in range(2)]
            kvout_kt = [p_dram.tile([S // 2, D], BF16, name=f"kvout_kt{i}") for i in range(2)]
            kvin_v = p_dram.tile([Q, D], BF16, name="kvin_v")
            kvout_v = p_dram.tile([S, D], BF16, name="kvout_v")

            # ---- constants / small tiles ----
            bqT = p_small.tile([128, NDC], F32)
            nc.gpsimd.dma_start(out=bqT[:, :], in_=bqT_d[:, :])
            bkT = p_small.tile([128, NDC], F32)
            nc.gpsimd.dma_start(out=bkT[:, :], in_=bkT_d[:, :])
            ones = p_small.tile([128, 1], BF16)
            nc.vector.memset(ones[:, :], 1.0)
            eps_t = p_small.tile([128, 1], F32)
            nc.vector.memset(eps_t[:, :], EPS)
            recip = p_small.tile([128, 8], F32)

            qt = [p_qt.tile([128, Q], BF16, tag="qt", name=f"qt{i}") for i in range(NDC)]
            pt = [p_pt.tile([128, Q], BF16, tag="pt", name=f"pt{i}") for i in range(NKC)]

            # ---- projections for own half: QT (sbuf), KT/V (to DRAM bounce) ----
            with (
                tc.tile_pool(name="wp", bufs=3 * NDC) as p_w,
                tc.tile_pool(name="xqtp", bufs=NDC) as p_xqt,
                tc.tile_pool(name="kvtmp", bufs=6) as p_kv,
            ):
                wq = [p_w.tile([128, D], BF16, tag="w", name=f"wq{i}") for i in range(NDC)]
                wk = [p_w.tile([128, D], BF16, tag="w", name=f"wk{i}") for i in range(NDC)]
                wv = [p_w.tile([128, D], BF16, tag="w", name=f"wv{i}") for i in range(NDC)]
                xqts = [p_xqt.tile([128, Q], BF16, tag="xqt", name=f"xqts{i}") for i in range(NDC)]
                for dc in range(NDC):
                    nc.sync.dma_start(out=xqts[dc][:, :], in_=xqt[128 * dc:128 * (dc + 1), :])
                    nc.gpsimd.dma_start(out=wk[dc][:, :], in_=wk_d[128 * dc:128 * (dc + 1), :])
                for dc in range(NDC):
                    nc.sync.dma_start(out=wq[dc][:, :], in_=wq_d[128 * dc:128 * (dc + 1), :])
                    nc.gpsimd.dma_start(out=wv[dc][:, :], in_=wv_d[128 * dc:128 * (dc + 1), :])

                # KT_own[d, k_own] (+bk) -> kvin_kt rows 128*do
                for do in range(NDC):
                    kt_t = p_kv.tile([128, Q], BF16, tag="kv", name=f"ktt{do}")
                    for kh in range(2):
                        ps = p_ps.tile([128, 512], F32, tag="ps")
                        for di in range(NDC):
                            nc.tensor.matmul(
                                ps[:, :],
                                wk[di][:, 128 * do:128 * (do + 1)],
                                xqts[di][:, 512 * kh:512 * (kh + 1)],
                                start=(di == 0), stop=(di == NDC - 1),
                            )
                        nc.vector.tensor_scalar(
                            out=kt_t[:, 512 * kh:512 * (kh + 1)], in0=ps[:, :],
                            scalar1=bkT[:, do:do + 1], scalar2=None,
                            op0=mybir.AluOpType.add,
                        )
                    half, dl = do // 4, do % 4
                    nc.sync.dma_start(
                        out=kvin_kt[half][128 * dl:128 * (dl + 1), :], in_=kt_t[:, :]
                    )
                    if do == 3 or do == 7:
                        nc.gpsimd.collective_compute(
                            "AllGather", mybir.AluOpType.bypass, replica_groups=RG,
                            ins=[kvin_kt[half][:, :].opt()],
                            outs=[kvout_kt[half][:, :].opt()],
                        )

                # QT[d,q]
                for do in range(NDC):
                    for qh in range(2):
                        ps = p_ps.tile([128, 512], F32, tag="ps")
                        for di in range(NDC):
                            nc.tensor.matmul(
                                ps[:, :],
                                wq[di][:, 128 * do:128 * (do + 1)],
                                xqts[di][:, 512 * qh:512 * (qh + 1)],
                                start=(di == 0), stop=(di == NDC - 1),
                            )
                        nc.vector.tensor_scalar(
                            out=qt[do][:, 512 * qh:512 * (qh + 1)], in0=ps[:, :],
                            scalar1=bqT[:, do:do + 1], scalar2=None,
                            op0=mybir.AluOpType.add,
                        )

                # V_own[k_own, d] -> kvin_v rows 128*kc
                for kl in range(NDC):
                    v_t = p_kv.tile([128, D], BF16, tag="kv", name=f"vt{kl}")
                    for dh in range(2):
                        ps = p_ps.tile([128, 512], F32, tag="ps")
                        for di in range(NDC):
                            nc.tensor.matmul(
                                ps[:, :],
                                xqts[di][:, 128 * kl:128 * (kl + 1)],
                                wv[di][:, 512 * dh:512 * (dh + 1)],
                                start=(di == 0), stop=(di == NDC - 1),
                            )
                        nc.vector.tensor_copy(v_t[:, 512 * dh:512 * (dh + 1)], ps[:, :])
                    nc.gpsimd.dma_start(out=kvin_v[128 * kl:128 * (kl + 1), :], in_=v_t[:, :])
                nc.gpsimd.collective_compute(
                    "AllGather", mybir.AluOpType.bypass, replica_groups=RG,
                    ins=[kvin_v[:, :].opt()], outs=[kvout_v[:, :].opt()],
                )

            # ---- load gathered KT (d-major per half); kbias; ST -> exp -> PT ----
            with tc.tile_pool(name="ktl", bufs=2 * NDC) as p_ktl:
                ktl = [[None] * NDC for _ in range(2)]
                for half in range(2):
                    for h in range(2):
                        for dl in range(4):
                            dc = 4 * half + dl
                            t = p_ktl.tile([128, Q], BF16, tag="ktl", name=f"ktl{h}_{dc}")
                            ktl[h][dc] = t
                            eng = nc.sync if (dl % 2 == 0) else nc.gpsimd
                            eng.dma_start(
                                out=t[:, :],
                                in_=kvout_kt[half][
                                    (Q // 2) * h + 128 * dl:(Q // 2) * h + 128 * (dl + 1), :
                                ],
                            )
                # ST -> exp -> PT
                for kc in range(NKC):
                    h, kcl = kc // NDC, kc % NDC
                    for qh in range(2):
                        ps = p_ps.tile([128, 512], F32, tag="ps")
                        for dc in range(NDC):
                            nc.tensor.matmul(
                                ps[:, :],
                                ktl[h][dc][:, 128 * kcl:128 * (kcl + 1)],
                                qt[dc][:, 512 * qh:512 * (qh + 1)],
                                start=(dc == 0), stop=(dc == NDC - 1),
                            )
                        nc.scalar.activation(
                            out=pt[kc][:, 512 * qh:512 * (qh + 1)], in_=ps[:, :],
                            func=mybir.ActivationFunctionType.Exp,
                            bias=0.0, scale=SCALE,
                        )

            # ---- AT[d,q] = V^T @ PT ; denom ; O ; LayerNorm ----
            with (
                tc.tile_pool(name="vtl", bufs=NKC) as p_vtl,
                tc.tile_pool(name="atp", bufs=NDC) as p_at,
            ):
                vtl = [p_vtl.tile([128, D], BF16, tag="vtl", name=f"vtl{i}") for i in range(NKC)]
                for kc in range(NKC):
                    eng = nc.sync if (kc % 2 == 0) else nc.gpsimd
                    eng.dma_start(
                        out=vtl[kc][:, :], in_=kvout_v[128 * kc:128 * (kc + 1), :]
                    )
                at = [p_at.tile([128, Q], BF16, tag="at", name=f"at{i}") for i in range(NDC)]
                for dc in range(NDC):
                    for qh in range(2):
                        ps = p_ps.tile([128, 512], F32, tag="ps")
                        for kc in range(NKC):
                            nc.tensor.matmul(
                                ps[:, :],
                                vtl[kc][:, 128 * dc:128 * (dc + 1)],
                                pt[kc][:, 512 * qh:512 * (qh + 1)],
                                start=(kc == 0), stop=(kc == NKC - 1),
                            )
                        nc.vector.tensor_copy(at[dc][:, 512 * qh:512 * (qh + 1)], ps[:, :])
                for qp in range(8):
                    ps1 = p_ps1.tile([128, 1], F32, tag="ps1")
                    for kc in range(NKC):
                        nc.tensor.matmul(
                            ps1[:, :],
                            pt[kc][:, 128 * qp:128 * (qp + 1)],
                            ones[:, :],
                            start=(kc == 0), stop=(kc == NKC - 1),
                        )
                    nc.vector.reciprocal(recip[:, qp:qp + 1], ps1[:, :])

                with (
                    tc.tile_pool(name="wop", bufs=NDC) as p_wo,
                    tc.tile_pool(name="xqp", bufs=3) as p_xq,
                    tc.tile_pool(name="vout", bufs=4) as p_vo,
                    tc.tile_pool(name="lnst", bufs=4) as p_ln,
                ):
                    wo = [p_wo.tile([128, D], BF16, tag="wo", name=f"wo{i}") for i in range(NDC)]
                    for dc in range(NDC):
                        nc.gpsimd.dma_start(out=wo[dc][:, :], in_=wo_d[128 * dc:128 * (dc + 1), :])
                    gam = p_small.tile([128, D], F32)
                    nc.gpsimd.dma_start(out=gam[:, :], in_=_bcast_ap(gamma_d[:]))
                    bet = p_small.tile([128, D], F32)
                    nc.gpsimd.dma_start(out=bet[:, :], in_=_bcast_ap(beta_d[:]))

                    for qp in range(8):
                        v = p_vo.tile([128, D], F32, tag="v")
                        sqs = p_vo.tile([128, D], F32, tag="sqs")
                        xqt_ = p_xq.tile([128, D], F32, tag="xq")
                        nc.sync.dma_start(
                            out=xqt_[:, :], in_=xq[128 * qp:128 * (qp + 1), :]
                        )
                        st = p_ln.tile([128, 4], F32, tag="st")
                        for eh in range(2):
                            ps = p_ps.tile([128, 512], F32, tag="ps")
                            for dc in range(NDC):
                                nc.tensor.matmul(
                                    ps[:, :],
                                    at[dc][:, 128 * qp:128 * (qp + 1)],
                                    wo[dc][:, 512 * eh:512 * (eh + 1)],
                                    start=(dc == 0), stop=(dc == NDC - 1),
                                )
                            # v_half = O/denom + xq_aug; accum = row-sum
                            nc.vector.scalar_tensor_tensor(
                                out=v[:, 512 * eh:512 * (eh + 1)], in0=ps[:, :],
                                scalar=recip[:, qp:qp + 1],
                                in1=xqt_[:, 512 * eh:512 * (eh + 1)],
                                op0=mybir.AluOpType.mult, op1=mybir.AluOpType.add,
                                accum_out=st[:, eh:eh + 1],
                            )
                        # E[v^2] via ACT Square + free accum; then mean/var/rstd
                        nc.scalar.activation(
                            out=sqs[:, :], in_=v[:, :],
                            func=mybir.ActivationFunctionType.Square,
                            accum_out=st[:, 2:3],
                        )
                        nc.vector.tensor_add(st[:, 0:1], st[:, 0:1], st[:, 1:2])
                        nc.scalar.mul(st[:, 0:1], st[:, 0:1], 1.0 / D)     # mean
                        nc.scalar.mul(st[:, 2:3], st[:, 2:3], 1.0 / D)     # E[v^2]
                        nc.vector.tensor_mul(st[:, 1:2], st[:, 0:1], st[:, 0:1])
                        nc.vector.tensor_sub(st[:, 2:3], st[:, 2:3], st[:, 1:2])
                        nc.scalar.activation(
                            out=st[:, 2:3], in_=st[:, 2:3],
                            func=mybir.ActivationFunctionType.Sqrt,
                            bias=eps_t[:, :],
                        )
                        nc.vector.reciprocal(st[:, 2:3], st[:, 2:3])       # rstd
                        # out = ((v - mean)*gamma)*rstd + beta  (2 fused DVE ops)
                        nc.vector.scalar_tensor_tensor(
                            out=v[:, :], in0=v[:, :], scalar=st[:, 0:1],
                            in1=gam[:, :],
                            op0=mybir.AluOpType.subtract, op1=mybir.AluOpType.mult,
                        )
                        nc.vector.scalar_tensor_tensor(
                            out=v[:, :], in0=v[:, :], scalar=st[:, 2:3],
                            in1=bet[:, :],
                            op0=mybir.AluOpType.mult, op1=mybir.AluOpType.add,
                        )
                        nc.sync.dma_start(out=out_d[128 * qp:128 * (qp + 1), :], in_=v[:, :])
    nc.compile()
    return nc


_NC_CACHE = None


def make_in_maps(inputs):
    x = np.asarray(inputs["inputs"], np.float32)
    wo = np.asarray(inputs["wo"], np.float32)
    bf = lambda a: np.ascontiguousarray(a).astype(ml_dtypes.bfloat16)
    bo_eff = np.asarray(inputs["bo"], np.float32) + np.asarray(inputs["bv"], np.float32) @ wo
    shared = {
        "wq": bf(inputs["wq"]), "wk": bf(inputs["wk"]),
        "wv": bf(inputs["wv"]), "wo": bf(wo),
        "bqT": np.ascontiguousarray(np.asarray(inputs["bq"], np.float32).reshape(NDC, 128).T),
        "bkT": np.ascontiguousarray(np.asarray(inputs["bk"], np.float32).reshape(NDC, 128).T),
        "gamma": np.asarray(inputs["gamma"], np.float32),
        "beta": np.asarray(inputs["beta"], np.float32),
    }
    in_maps = []
    for c in range(8):
        b, qh = c // 2, c % 2
        xslab = x[b, Q * qh:Q * (qh + 1), :]
        in_maps.append({
            **shared,
            "xqt": bf(xslab.T),
            "xq": np.ascontiguousarray(xslab) + bo_eff[None, :],
        })
    return in_maps


def kernel(**inputs) -> np.ndarray:
    from concourse.bass_utils import run_bass_kernel_spmd

    global _NC_CACHE
    if _NC_CACHE is None:
        _NC_CACHE = build_nc()
    res = run_bass_kernel_spmd(_NC_CACHE, make_in_maps(inputs), core_ids=list(range(8)))
    out = np.empty((B, S, D), np.float32)
    for c in range(8):
        b, qh = c // 2, c % 2
        out[b, Q * qh:Q * (qh + 1), :] = res.results[c]["out"]
    return out


# revision 13
# speedup vs baseline: 1.1749x; 1.1749x over previous
"""Trainium2 Bass kernel for full-embed-dim self-attention + residual LayerNorm.

Problem: B=4, S=2048, D=1024 fp32.
  q/k/v = x@w{q,k,v}+b; scores = q@k^T/sqrt(D); attn = softmax(scores)@v;
  out = LN(x + attn@wo + bo) * gamma + beta.

Sharding: 8 cores = 4 batches x 2 query-halves (1024 queries each). Each
core computes K/V projections only for its own 1024 keys, then a pair-wise
AllGather ([0,1],[2,3],...) exchanges the halves so each core attends over
the batch's full 2048-key sequence.

Per-core dataflow (d-on-partitions "transposed" layout throughout):
  QT[d,q]   = wq^T @ xq^T            (host ships x^T for the core's half)
  KT[d,k_own] = wk^T @ xq^T (+bk per-partition)   --> AllGather --> KT full
  V[k_own,d]  = xq^T^T @ wv                       --> AllGather --> V full
  ST[k,q]   = KT^T @ QT              (contracted over d in PSUM)
  PT[k,q]   = exp(ST/sqrt(D) + kbias[k]),  kbias = K@(bq/sqrt(D))
              (per-q softmax factors cancel after normalization; logits are
               O(5) std-1 so exp needs no max-subtraction in fp32)
  denom[q]  = ones^T @ PT            (PE column-sum)
  AT[d,q]   = V^T @ PT
  O[q,e]    = AT^T @ wo, then O/denom[q] + xq_aug (host adds bo+bv@wo), LN.

Matmul inputs are bf16 (1 PE cycle/row vs 4 for fp32; fp32 PSUM accum).
"""

import numpy as np
import ml_dtypes

import concourse.bass as bass
import concourse.mybir as mybir
import concourse.tile as tile
from concourse import bacc

F32 = mybir.dt.float32
BF16 = mybir.dt.bfloat16

B, S, D = 4, 2048, 1024
Q = 1024          # queries (and own keys) per core
SCALE = 1.0 / 32.0
EPS = 1e-6
NKC = S // 128    # 16 key chunks (global)
NDC = D // 128    # 8 d chunks
RG = [[0, 1], [2, 3], [4, 5], [6, 7]]


def _bcast_ap(ap_1d, parts=128):
    """[N] dram AP -> [parts, N] AP with 0-stride partition dim."""
    return bass.AP(
        tensor=ap_1d.tensor, offset=ap_1d.offset, ap=[[0, parts]] + list(ap_1d.ap)
    )


def build_nc():
    nc = bacc.Bacc("TRN2", target_bir_lowering=False, debug=False, num_devices=8)

    xqt = nc.dram_tensor("xqt", [D, Q], BF16, kind="ExternalInput")
    xq = nc.dram_tensor("xq", [Q, D], F32, kind="ExternalInput")
    wq_d = nc.dram_tensor("wq", [D, D], BF16, kind="ExternalInput")
    wk_d = nc.dram_tensor("wk", [D, D], BF16, kind="ExternalInput")
    wv_d = nc.dram_tensor("wv", [D, D], BF16, kind="ExternalInput")
    wo_d = nc.dram_tensor("wo", [D, D], BF16, kind="ExternalInput")
    bqT_d = nc.dram_tensor("bqT", [128, NDC], F32, kind="ExternalInput")
    bkT_d = nc.dram_tensor("bkT", [128, NDC], F32, kind="ExternalInput")
    gamma_d = nc.dram_tensor("gamma", [D], F32, kind="ExternalInput")
    beta_d = nc.dram_tensor("beta", [D], F32, kind="ExternalInput")
    out_d = nc.dram_tensor("out", [Q, D], F32, kind="ExternalOutput")

    with tile.TileContext(nc) as tc:
        with (
            tc.tile_pool(name="small", bufs=1) as p_small,
            tc.tile_pool(name="dram", bufs=1, space="DRAM") as p_dram,
            tc.tile_pool(name="qtsb", bufs=NDC) as p_qt,
            tc.tile_pool(name="ptsb", bufs=NKC) as p_pt,
            tc.tile_pool(name="kto", bufs=NDC) as p_kto,
            tc.tile_pool(name="vow", bufs=NDC) as p_vow,
        ):
            kvin_kt = p_dram.tile([Q, D], BF16, name="kvin_kt")
            kvout_kt = p_dram.tile([S, D], BF16, name="kvout_kt")
            kvin_v = p_dram.tile([Q, D], BF16, name="kvin_v")
            kvout_v = p_dram.tile([S, D], BF16, name="kvout_v")

            # ---- constants / small tiles ----
            bqT = p_small.tile([128, NDC], F32)
            nc.gpsimd.dma_start(out=bqT[:, :], in_=bqT_d[:, :])
            bkT = p_small.tile([128, NDC], F32)
            nc.gpsimd.dma_start(out=bkT[:, :], in_=bkT_d[:, :])
            ones = p_small.tile([128, 1], BF16)
            nc.vector.memset(ones[:, :], 1.0)
            eps_t = p_small.tile([128, 1], F32)
            nc.vector.memset(eps_t[:, :], EPS)
            recip = p_small.tile([128, 8], F32)

            pid = nc.sync.partition_id()
            partner_off = (1 - (pid % 2)) * Q   # partner's row base in gathered buffers

            qt = [p_qt.tile([128, Q], BF16, tag="qt", name=f"qt{i}") for i in range(NDC)]
            pt = [p_pt.tile([128, Q], BF16, tag="pt", name=f"pt{i}") for i in range(NKC)]

            # ---- projections for own half: QT (sbuf), KT/V (to DRAM bounce) ----
            with (
                tc.tile_pool(name="wp", bufs=3 * NDC) as p_w,
                tc.tile_pool(name="xqtp", bufs=NDC) as p_xqt,
                            ):
                wq = [p_w.tile([128, D], BF16, tag="w", name=f"wq{i}") for i in range(NDC)]
                wk = [p_w.tile([128, D], BF16, tag="w", name=f"wk{i}") for i in range(NDC)]
                wv = [p_w.tile([128, D], BF16, tag="w", name=f"wv{i}") for i in range(NDC)]
                xqts = [p_xqt.tile([128, Q], BF16, tag="xqt", name=f"xqts{i}") for i in range(NDC)]
                for dc in range(NDC):
                    nc.sync.dma_start(out=xqts[dc][:, :], in_=xqt[128 * dc:128 * (dc + 1), :])
                    nc.gpsimd.dma_start(out=wk[dc][:, :], in_=wk_d[128 * dc:128 * (dc + 1), :])
                for dc in range(NDC):
                    nc.sync.dma_start(out=wq[dc][:, :], in_=wq_d[128 * dc:128 * (dc + 1), :])
                    nc.gpsimd.dma_start(out=wv[dc][:, :], in_=wv_d[128 * dc:128 * (dc + 1), :])

                # Projections run di (contraction) outermost over 8 PSUM banks so
                # the PE starts as soon as the first (wk, xqt) chunk pair lands.
                with tc.tile_pool(name="psp", bufs=8, space="PSUM") as p_psp:
                    # KT_own[d, k_own] (+bk) -> kvin_kt rows 128*do
                    kt_ts = [p_kto.tile([128, Q], BF16, tag="kto", name=f"ktt{do}") for do in range(NDC)]
                    for kh in range(2):
                        pss = [p_psp.tile([128, 512], F32, tag="psp", name=f"pskt{kh}_{do}") for do in range(NDC)]
                        for di in range(NDC):
                            for do in range(NDC):
                                nc.tensor.matmul(
                                    pss[do][:, :],
                                    wk[di][:, 128 * do:128 * (do + 1)],
                                    xqts[di][:, 512 * kh:512 * (kh + 1)],
                                    start=(di == 0), stop=(di == NDC - 1),
                                )
                        for do in range(NDC):
                            nc.vector.tensor_scalar(
                                out=kt_ts[do][:, 512 * kh:512 * (kh + 1)], in0=pss[do][:, :],
                                scalar1=bkT[:, do:do + 1], scalar2=None,
                                op0=mybir.AluOpType.add,
                            )
                    for do in range(NDC):
                        nc.sync.dma_start(out=kvin_kt[128 * do:128 * (do + 1), :], in_=kt_ts[do][:, :])
                    nc.gpsimd.collective_compute(
                        "AllGather", mybir.AluOpType.bypass, replica_groups=RG,
                        ins=[kvin_kt[:, :].opt()], outs=[kvout_kt[:, :].opt()],
                    )

                    # QT[d,q]
                    for qh in range(2):
                        pss = [p_psp.tile([128, 512], F32, tag="psp", name=f"psqt{qh}_{do}") for do in range(NDC)]
                        for di in range(NDC):
                            for do in range(NDC):
                                nc.tensor.matmul(
                                    pss[do][:, :],
                                    wq[di][:, 128 * do:128 * (do + 1)],
                                    xqts[di][:, 512 * qh:512 * (qh + 1)],
                                    start=(di == 0), stop=(di == NDC - 1),
                                )
                        for do in range(NDC):
                            nc.vector.tensor_scalar(
                                out=qt[do][:, 512 * qh:512 * (qh + 1)], in0=pss[do][:, :],
                                scalar1=bqT[:, do:do + 1], scalar2=None,
                                op0=mybir.AluOpType.add,
                            )

                    # V_own[k_own, d] -> kvin_v rows 128*kc
                    v_ts = [p_vow.tile([128, D], BF16, tag="vow", name=f"vt{kl}") for kl in range(NDC)]
                    for dh in range(2):
                        pss = [p_psp.tile([128, 512], F32, tag="psp", name=f"psv{dh}_{kl}") for kl in range(NDC)]
                        for di in range(NDC):
                            for kl in range(NDC):
                                nc.tensor.matmul(
                                    pss[kl][:, :],
                                    xqts[di][:, 128 * kl:128 * (kl + 1)],
                                    wv[di][:, 512 * dh:512 * (dh + 1)],
                                    start=(di == 0), stop=(di == NDC - 1),
                                )
                        for kl in range(NDC):
                            nc.vector.tensor_copy(v_ts[kl][:, 512 * dh:512 * (dh + 1)], pss[kl][:, :])
                    for kl in range(NDC):
                        nc.gpsimd.dma_start(out=kvin_v[128 * kl:128 * (kl + 1), :], in_=v_ts[kl][:, :])
                    nc.gpsimd.collective_compute(
                        "AllGather", mybir.AluOpType.bypass, replica_groups=RG,
                        ins=[kvin_v[:, :].opt()], outs=[kvout_v[:, :].opt()],
                    )

            # ---- load gathered KT (d-major per half); ST -> exp -> PT ----
            with (
                tc.tile_pool(name="ps", bufs=6, space="PSUM") as p_ps,
                tc.tile_pool(name="ps1", bufs=2, space="PSUM") as p_ps1,
                tc.tile_pool(name="ktl", bufs=NDC) as p_ktl,
                tc.tile_pool(name="vtl", bufs=NDC) as p_vtl,
                tc.tile_pool(name="atp", bufs=NDC) as p_at,
            ):
                # partner-half KT via runtime-parity offset into the gathered buffer
                ktl = [None] * NDC
                for dc in range(NDC):
                    t = p_ktl.tile([128, Q], BF16, tag="ktl", name=f"ktl{dc}")
                    ktl[dc] = t
                    nc.sync.dma_start(
                        out=t[:, :],
                        in_=kvout_kt[bass.ds(partner_off + 128 * dc, 128), :],
                    )
                # ST -> exp -> PT; PT in LOCAL key order: kc 0..7 = own half
                # (from SBUF, no collective dependency), kc 8..15 = partner.
                # Softmax/attention sums over k are order-invariant as long as
                # the V tiles use the same local order.
                for kc in range(NKC):
                    own, kcl = kc < NDC, kc % NDC
                    for qh in range(2):
                        ps = p_ps.tile([128, 512], F32, tag="ps")
                        for dc in range(NDC):
                            lhs = kt_ts[dc] if own else ktl[dc]
                            nc.tensor.matmul(
                                ps[:, :],
                                lhs[:, 128 * kcl:128 * (kcl + 1)],
                                qt[dc][:, 512 * qh:512 * (qh + 1)],
                                start=(dc == 0), stop=(dc == NDC - 1),
                            )
                        nc.scalar.activation(
                            out=pt[kc][:, 512 * qh:512 * (qh + 1)], in_=ps[:, :],
                            func=mybir.ActivationFunctionType.Exp,
                            bias=0.0, scale=SCALE,
                        )

                # ---- AT[d,q] = V^T @ PT ; denom ; O ; LayerNorm ----
                vtl = [p_vtl.tile([128, D], BF16, tag="vtl", name=f"vtl{i}") for i in range(NDC)]
                for kl in range(NDC):
                    nc.sync.dma_start(
                        out=vtl[kl][:, :],
                        in_=kvout_v[bass.ds(partner_off + 128 * kl, 128), :],
                    )
                at = [p_at.tile([128, Q], BF16, tag="at", name=f"at{i}") for i in range(NDC)]
                for dc in range(NDC):
                    for qh in range(2):
                        ps = p_ps.tile([128, 512], F32, tag="ps")
                        for kc in range(NKC):
                            vt = v_ts[kc] if kc < NDC else vtl[kc - NDC]
                            nc.tensor.matmul(
                                ps[:, :],
                                vt[:, 128 * dc:128 * (dc + 1)],
                                pt[kc][:, 512 * qh:512 * (qh + 1)],
                                start=(kc == 0), stop=(kc == NKC - 1),
                            )
                        nc.vector.tensor_copy(at[dc][:, 512 * qh:512 * (qh + 1)], ps[:, :])
                for qp in range(8):
                    ps1 = p_ps1.tile([128, 1], F32, tag="ps1")
                    for kc in range(NKC):
                        nc.tensor.matmul(
                            ps1[:, :],
                            pt[kc][:, 128 * qp:128 * (qp + 1)],
                            ones[:, :],
                            start=(kc == 0), stop=(kc == NKC - 1),
                        )
                    nc.vector.reciprocal(recip[:, qp:qp + 1], ps1[:, :])

                with (
                    tc.tile_pool(name="wop", bufs=NDC) as p_wo,
                    tc.tile_pool(name="xqp", bufs=3) as p_xq,
                    tc.tile_pool(name="vout", bufs=4) as p_vo,
                    tc.tile_pool(name="lnst", bufs=4) as p_ln,
                ):
                    wo = [p_wo.tile([128, D], BF16, tag="wo", name=f"wo{i}") for i in range(NDC)]
                    for dc in range(NDC):
                        nc.gpsimd.dma_start(out=wo[dc][:, :], in_=wo_d[128 * dc:128 * (dc + 1), :])
                    gam = p_small.tile([128, D], F32)
                    nc.gpsimd.dma_start(out=gam[:, :], in_=_bcast_ap(gamma_d[:]))
                    bet = p_small.tile([128, D], F32)
                    nc.gpsimd.dma_start(out=bet[:, :], in_=_bcast_ap(beta_d[:]))

                    for qp in range(8):
                        v = p_vo.tile([128, D], F32, tag="v")
                        sqs = p_vo.tile([128, D], F32, tag="sqs")
                        xqt_ = p_xq.tile([128, D], F32, tag="xq")
                        nc.gpsimd.dma_start(
                            out=xqt_[:, :], in_=xq[128 * qp:128 * (qp + 1), :]
                        )
                        st = p_ln.tile([128, 4], F32, tag="st")
                        for eh in range(2):
                            ps = p_ps.tile([128, 512], F32, tag="ps")
                            for dc in range(NDC):
                                nc.tensor.matmul(
                                    ps[:, :],
                                    at[dc][:, 128 * qp:128 * (qp + 1)],
                                    wo[dc][:, 512 * eh:512 * (eh + 1)],
                                    start=(dc == 0), stop=(dc == NDC - 1),
                                )
                            # v_half = O/denom + xq_aug; accum = row-sum
                            nc.vector.scalar_tensor_tensor(
                                out=v[:, 512 * eh:512 * (eh + 1)], in0=ps[:, :],
                                scalar=recip[:, qp:qp + 1],
                                in1=xqt_[:, 512 * eh:512 * (eh + 1)],
                                op0=mybir.AluOpType.mult, op1=mybir.AluOpType.add,
                                accum_out=st[:, eh:eh + 1],
                            )
                        # E[v^2] via ACT Square + free accum; then mean/var/rstd
                        nc.scalar.activation(
                            out=sqs[:, :], in_=v[:, :],
                            func=mybir.ActivationFunctionType.Square,
                            accum_out=st[:, 2:3],
                        )
                        nc.vector.tensor_add(st[:, 0:1], st[:, 0:1], st[:, 1:2])
                        nc.scalar.mul(st[:, 0:1], st[:, 0:1], 1.0 / D)     # mean
                        nc.scalar.mul(st[:, 2:3], st[:, 2:3], 1.0 / D)     # E[v^2]
                        nc.vector.tensor_mul(st[:, 1:2], st[:, 0:1], st[:, 0:1])
                        nc.vector.tensor_sub(st[:, 2:3], st[:, 2:3], st[:, 1:2])
                        nc.scalar.activation(
                            out=st[:, 2:3], in_=st[:, 2:3],
                            func=mybir.ActivationFunctionType.Sqrt,
                            bias=eps_t[:, :],
                        )
                        nc.vector.reciprocal(st[:, 2:3], st[:, 2:3])       # rstd
                        # out = ((v - mean)*gamma)*rstd + beta  (2 fused DVE ops)
                        nc.vector.scalar_tensor_tensor(
                            out=v[:, :], in0=v[:, :], scalar=st[:, 0:1],
                            in1=gam[:, :],
                            op0=mybir.AluOpType.subtract, op1=mybir.AluOpType.mult,
                        )
                        nc.vector.scalar_tensor_tensor(
                            out=v[:, :], in0=v[:, :], scalar=st[:, 2:3],
                            in1=bet[:, :],
                            op0=mybir.AluOpType.mult, op1=mybir.AluOpType.add,
                        )
                        nc.sync.dma_start(out=out_d[128 * qp:128 * (qp + 1), :], in_=v[:, :])
    nc.compile()
    return nc


_NC_CACHE = None


def make_in_maps(inputs):
    x = np.asarray(inputs["inputs"], np.float32)
    wo = np.asarray(inputs["wo"], np.float32)
    bf = lambda a: np.ascontiguousarray(a).astype(ml_dtypes.bfloat16)
    bo_eff = np.asarray(inputs["bo"], np.float32) + np.asarray(inputs["bv"], np.float32) @ wo
    shared = {
        "wq": bf(inputs["wq"]), "wk": bf(inputs["wk"]),
        "wv": bf(inputs["wv"]), "wo": bf(wo),
        "bqT": np.ascontiguousarray(np.asarray(inputs["bq"], np.float32).reshape(NDC, 128).T),
        "bkT": np.ascontiguousarray(np.asarray(inputs["bk"], np.float32).reshape(NDC, 128).T),
        "gamma": np.asarray(inputs["gamma"], np.float32),
        "beta": np.asarray(inputs["beta"], np.float32),
    }
    in_maps = []
    for c in range(8):
        b, qh = c // 2, c % 2
        xslab = x[b, Q * qh:Q * (qh + 1), :]
        in_maps.append({
            **shared,
            "xqt": bf(xslab.T),
            "xq": np.ascontiguousarray(xslab) + bo_eff[None, :],
        })
    return in_maps


def kernel(**inputs) -> np.ndarray:
    from concourse.bass_utils import run_bass_kernel_spmd

    global _NC_CACHE
    if _NC_CACHE is None:
        _NC_CACHE = build_nc()
    res = run_bass_kernel_spmd(_NC_CACHE, make_in_maps(inputs), core_ids=list(range(8)))
    out = np.empty((B, S, D), np.float32)
    for c in range(8):
        b, qh = c // 2, c % 2
        out[b, Q * qh:Q * (qh + 1), :] = res.results[c]["out"]
    return out
